# revision 39
# baseline (speedup 1.0000x reference)
"""Laplace attention kernel for Trainium2 (8 NeuronCores, SPMD data-parallel).

Reference computation (per batch b):
    unnorm[i,j] = sum_d |(k[j,d] - v[i,d]) * 0.5|
    weights     = softmax_j(unnorm)          # rows i, softmax over j
    out[i,:]    = sum_j weights[i,j] * v[j,:]

B=8 batches -> one batch per NeuronCore, no cross-core communication.

Per-core algorithm (M=512, D=64, P=128):
  - Layouts:  vT2 [128=(t,d), 512=i] fp16 : v transposed, duplicated over t
              k2T [128=(t,d), 256=mj] f32 : column mj = [k[2mj,:]; k[2mj+1,:]]
  - For each j-pair mj: one DVE tensor_scalar
        absd[(t,d), i] = max(vT2, k2T[:,mj]) = max(v[i,d], k[2mj+t,d])
    then one TensorE matmul with a constant selector lhsT [128,2]
    (column t selects the 64 d-rows of half t) reducing over d:
        unnT[2m+t, i] += ... -> PSUM bank q holds unnT rows j=128q..128q+127
    |a-b| = 2*max(a,b) - a - b; the V1[i] part cancels in the softmax and
    the K1[j] part folds into the exp bias.  unnT is produced TRANSPOSED
    ([j,i]) which is exactly the lhsT the final matmul needs.
  - Producer split: banks 0..2 on VectorE (tensor_scalar max), bank 3 on
    ScalarE as Relu(v - k) = max(v,k) - k (bias absorbs the K1 sign flip).
    PE stream: 64 groups of (3 DVE-fed + 1 ScalarE-prebuffered) matmuls.
  - Softmax numerators wT[j,i] = exp(unnT - 0.5*K1[j] - SHIFT) in bf16.
  - Final matmul with v augmented by a ones column gives numerator and
    denominator together; one strided reciprocal + one broadcast multiply
    normalizes all 4 row-blocks; single DMA out.

Edge scheduling (v2): input DMAs issued from the gpsimd (k) and
vector+scalar (v halves) queues which come up earliest; PE warmed by dummy
matmuls on a memset tile until the transposes can start; all drain work
single-shot to cut the serial tail.
"""

import os

import numpy as np

M = 512
D = 64
B = 8
P = 128
NB = M // P  # 4 row-blocks
NMJ = M // 2  # 256 j-pairs
# Global shift on the softmax logits: weights are stored as
# exp(logit - EXP_SHIFT); numerator and denominator scale identically.
EXP_SHIFT = 38.0

_CACHE = {}

CFG = {"mx_dt": "float16"}


def _build_module(cfg=None):
    import concourse.mybir as mybir
    import concourse.tile as tile
    from concourse import bacc

    nc = bacc.Bacc("TRN2", target_bir_lowering=False, debug=False,
                   enable_asserts=False)
    k_dram = nc.dram_tensor("k", [M, D], mybir.dt.float32, kind="ExternalInput")
    v_dram = nc.dram_tensor("v", [M, D], mybir.dt.float32, kind="ExternalInput")
    out_dram = nc.dram_tensor("out", [M, D], mybir.dt.float32,
                              kind="ExternalOutput")

    with tile.TileContext(nc) as tc:
        _emit(tc, nc, k_dram.ap(), v_dram.ap(), out_dram.ap(), cfg or CFG)
    nc.compile()
    return nc


def _emit(tc, nc, k, v, out, cfg):
    from contextlib import ExitStack

    import concourse.mybir as mybir
    from concourse.masks import make_identity

    f32 = mybir.dt.float32
    fp16 = getattr(mybir.dt, cfg.get("mx_dt", "float16"))
    bf16 = mybir.dt.bfloat16
    Alu = mybir.AluOpType
    Act = mybir.ActivationFunctionType

    ctx = ExitStack()
    const = ctx.enter_context(tc.tile_pool(name="const", bufs=1))
    # Deep rings: DVE produces at ~262 ns/tile, PE consumes at ~200; buffering
    # lets the PE run at its native rate.  ScalarE prebuffers all of bank 3.
    absd_pool = ctx.enter_context(tc.tile_pool(name="absd", bufs=48))
    act_pool = ctx.enter_context(tc.tile_pool(name="absd_act", bufs=66))
    wt_pool = ctx.enter_context(tc.tile_pool(name="wt", bufs=4))
    small = ctx.enter_context(tc.tile_pool(name="small", bufs=1))
    # All three PSUM pools stay alive together (3 + 4 + 1 = 8 banks), so
    # the main accumulation never waits on the transpose pool's release.
    psum_tr = ctx.enter_context(tc.tile_pool(name="psum_tr", bufs=1,
                                             space="PSUM"))

    # ---- input DMAs first, at top scheduling priority --------------------
    # k whole on the gpsimd SWDGE path (issues earliest, 256 fat
    # descriptors, lands ~0.5us before v); v split across the two HWDGE
    # queues so its 512 thin descriptors transfer in parallel.
    v4 = const.tile([P, NB, D], f32, name="v4")
    v_view = v.rearrange("(q p) d -> p q d", p=P)
    k2_view = k.rearrange("(h m t) d -> m h (t d)", t=2, h=2)  # [128, 2, 128]
    k2all = const.tile([P, 2, P], f32, name="k2all")
    with tc.high_priority():
        nc.sync.dma_start(k2all[:], k2_view[:])
        nc.scalar.dma_start(v4[:, 0:2, :], v_view[:, 0:2, :])
        nc.sync.dma_start(v4[:, 2:4, :], v_view[:, 2:4, :])

    # ---- warm-up source for PE p-state ramp ------------------------------
    warm_src = const.tile([P, P], fp16, name="warm_src")
    nc.gpsimd.memset(warm_src[:], 0.0)

    # ---- static tensors (gpsimd, after the DMA issues) -------------------
    # band[c, y] = 1 iff y == 64 + (c >= 64).  lhsT for local pair m is
    # band[:, 64-2m : 128-2m]: column p is 1 exactly when p == 2m + t(c).
    band = const.tile([P, 132], fp16, name="band")
    nc.gpsimd.memset(band[:], 0.0)
    nc.gpsimd.memset(band[0:D, D:D + 1], 1.0)
    nc.gpsimd.memset(band[D:2 * D, D + 1:D + 2], 1.0)
    ident = const.tile([P, P], f32, name="ident")
    make_identity(nc, ident)
    ident16 = const.tile([P, P], fp16, name="ident16")
    make_identity(nc, ident16)

    # ---- PE p-state warmup while DMAs are in flight ----------------------
    warm = psum_tr.tile([1, P], f32, name="warm", tag="warm")
    for _ in range(8):
        nc.tensor.matmul(warm[:], warm_src[:, 0:1], warm_src[:, 0:P],
                         start=True, stop=True)

    # ---- vT2 [128=(t,d), 512=i] fp16 (v lands first; PE does v first) ----
    # Everything split per DMA half / i-half so cast, transpose and the
    # duplicating copies pipeline instead of serializing.
    vT2 = const.tile([P, M], fp16, name="vT2")
    v16 = const.tile([P, NB, D], fp16, name="v16")
    ptv = psum_tr.tile([D, M], fp16, name="ptv", tag="ptv")
    for half in range(2):
        q0 = 2 * half
        nc.vector.tensor_copy(
            v16[:, q0:q0 + 2, :].rearrange("p q d -> p (q d)"),
            v4[:, q0:q0 + 2, :].rearrange("p q d -> p (q d)"))
        for q in (q0, q0 + 1):
            nc.tensor.transpose(ptv[:, q * P:(q + 1) * P], v16[:, q, :],
                                ident16[:])
    for half in range(2):
        sl = slice(half * 2 * P, (half + 1) * 2 * P)
        nc.vector.tensor_copy(vT2[0:D, sl], ptv[:, sl])
        nc.vector.tensor_copy(vT2[D:2 * D, sl], ptv[:, sl])

    # ---- k2T [128=(t,d), 256=mj] f32 -------------------------------------
    # Low half (banks 0,1 scalars for DVE) copied on DVE, high half on
    # ScalarE which then only negates the bank-3 columns it needs for the
    # Relu bias before starting its Relu stream.  tile_wait_until keeps the
    # scheduler from parking the k transposes at the PE queue head where
    # they would block the earlier-ready v transposes on the late k DMA.
    k2T = const.tile([P, NMJ], f32, name="k2T")
    ptrk = psum_tr.tile([P, 2 * P], f32, name="ptrk", tag="ptrk")
    with tc.tile_wait_until(0.0050):
        for h in range(2):
            nc.tensor.transpose(ptrk[:, h * P:(h + 1) * P], k2all[:, h, :],
                                ident[:])
        nc.vector.tensor_copy(k2T[:, 0:P], ptrk[:, 0:P])
        nc.scalar.copy(k2T[:, P:2 * P], ptrk[:, P:2 * P])
        neg_k2T = const.tile([P, D], f32, name="neg_k2T")  # mj 192..255
        nc.scalar.mul(neg_k2T[:], k2T[:, 3 * D:4 * D], -1.0)

    # ---- K1[j] = sum_d k[j,d] --------------------------------------------
    # Computed on ScalarE via activation accum_out (emitted inside the Relu
    # stream below, where ScalarE has slack); tiny scatter DMAs then build
    # the j-major per-bank bias columns.  Nothing touches DVE's hot path.
    k1m = const.tile([P, 2, 2], f32, name="k1m")
    k1scr = const.tile([P, D], fp16, name="k1scr")

    def emit_k1():
        for h in range(2):
            for t in range(2):
                nc.scalar.activation(k1scr[:], k2all[:, h, t * D:(t + 1) * D],
                                     Act.Copy, accum_out=k1m[:, h, t:t + 1])
        for q in range(NB):
            nc.sync.dma_start(k1_cols[:, q:q + 1],
                              k1m[(q % 2) * D:(q % 2) * D + D, q // 2, :])

    k1_cols = const.tile([P, NB], f32, name="k1_cols")

    # ---- main-phase PSUM pools -------------------------------------------
    psum_unn = ctx.enter_context(tc.tile_pool(name="psum_unn", bufs=4,
                                              space="PSUM"))
    psum_out = ctx.enter_context(tc.tile_pool(name="psum_out", bufs=1,
                                              space="PSUM"))
    out_all = psum_out.tile([P, NB, D + 1], f32, name="out_all")
    unns = [None] * NB
    for q in range(NB):
        unns[q] = psum_unn.tile([P, M], f32, name=f"unn_{q}", tag="unn")

    bias_col = [None] * NB
    wts = [None] * NB

    def emit_bias():
        for q in range(NB):
            bc = const.tile([P, 1], f32, name=f"bias_{q}")
            sgn = 0.5 if q == NB - 1 else -0.5  # bank 3 is the Relu path
            nc.scalar.activation(bc[:], k1_cols[:, q:q + 1], Act.Copy,
                                 bias=-EXP_SHIFT, scale=sgn)
            bias_col[q] = bc

    v_aug = []

    def emit_v_aug():
        for q in range(NB):
            va = const.tile([P, D + 1], bf16, name=f"v_aug_{q}")
            nc.scalar.copy(va[:, 0:D], v4[:, q, :])
            nc.gpsimd.memset(va[:, D:D + 1], 1.0)
            v_aug.append(va)

    def emit_exp(q, chunks=1):
        wT = wt_pool.tile([P, M], bf16, name="wT", tag="wT")
        wts[q] = wT
        cw = M // chunks
        for c in range(chunks):
            nc.scalar.activation(wT[:, c * cw:(c + 1) * cw],
                                 unns[q][:, c * cw:(c + 1) * cw],
                                 Act.Exp, scale=1.0, bias=bias_col[q][:])

    # ---- bank-3 distance tiles: ScalarE emits all 64 up front, with the
    # drain-phase helpers slotted into its queue where they have slack ----
    absd_a_tiles = {}
    for step in range(64):
        h, m = step % 2, step // 2
        mjl = h * 32 + m                # local col within the bank-3 block
        absd = act_pool.tile([P, M], fp16, name="absd_a", tag="absd_a")
        nc.scalar.activation(absd[:], vT2[:], Act.Relu,
                             bias=neg_k2T[:, mjl:mjl + 1], scale=1.0)
        absd_a_tiles[step] = absd
        if step == 3:
            emit_v_aug()
        elif step == 6:
            emit_k1()
        elif step == 16:
            emit_bias()

    def emit_step(q, step, absd):
        h, m = step % 2, step // 2
        nc.tensor.matmul(
            unns[q][D * h:D * h + D, :], band[:, D - 2 * m:2 * D - 2 * m],
            absd[:], start=(m == 0), stop=(m == 31), skip_group_check=True)

    # PE stream: groups of (3 VectorE-fed + 1 ScalarE-prebuffered) matmuls.
    # First group is all-DVE (ScalarE's first Relu tile lands late in the
    # startup chain).  The last group is all-DVE with group 62 taking three
    # prebuffered ScalarE tiles, so bank 3 closes ~0.8us before the stream
    # ends and its exp overlaps the last matmuls.
    sched = ["D"] * 4
    for g in range(1, 62):
        sched += ["D", "D", "D", "S"]
    sched += ["D", "S", "S", "S"]
    sched += ["D"] * 4
    ds = 0
    ss = 0
    for kind in sched:
        if kind == "D":
            q, step = ds // 64, ds % 64
            ds += 1
            h, m = step % 2, step // 2
            mj = q * 64 + h * 32 + m
            absd = absd_pool.tile([P, M], fp16, name="absd", tag="absd")
            nc.vector.tensor_scalar(
                absd[:], vT2[:], k2T[:, mj:mj + 1], None, op0=Alu.max)
            emit_step(q, step, absd)
        else:
            emit_step(NB - 1, ss, absd_a_tiles[ss])
            ss += 1

    # ---- softmax numerators ----------------------------------------------
    # Bank 3 closes early (schedule above) so its exp overlaps the last
    # matmuls; bank 2's exp is the only one that trails the stream.
    emit_exp(0)
    emit_exp(1)
    emit_exp(3)
    emit_exp(2)

    # ---- weighted sum + denominator via augmented-ones column ------------
    for qp in range(NB):
        for q in range(NB):
            nc.tensor.matmul(
                out_all[:, qp, :], wts[q][:, qp * P:(qp + 1) * P],
                v_aug[q][:], start=(q == 0), stop=(q == NB - 1),
                skip_group_check=True)

    # ---- normalize (single shot) + single DMA out ------------------------
    recip = small.tile([P, NB], f32, name="recip")
    nc.vector.reciprocal(recip[:], out_all[:, :, D])
    res = small.tile([P, NB, D], f32, name="res")
    rb = recip[:].unsqueeze(2).broadcast_to((P, NB, D))
    nc.vector.tensor_tensor(res[:], out_all[:, :, 0:D], rb, op=Alu.mult)
    out_v = out.rearrange("(q p) d -> p q d", p=P)
    nc.sync.dma_start(out_v[:, 0:2, :], res[:, 0:2, :])
    nc.scalar.dma_start(out_v[:, 2:4, :], res[:, 2:4, :])

    ctx.close()


def _get_module():
    if "nc" not in _CACHE:
        _CACHE["nc"] = _build_module()
    return _CACHE["nc"]


def _run(k, v, trace=False, tmpdir=None):
    """k, v: [B, M, D] f32. Returns (out [B, M, D] f32, BassKernelResults)."""
    from concourse import bass_utils

    nc = _get_module()
    kw = {"tmpdir": tmpdir} if tmpdir else {}
    in_maps = [
        {"k": np.ascontiguousarray(k[b], dtype=np.float32),
         "v": np.ascontiguousarray(v[b], dtype=np.float32)}
        for b in range(B)
    ]
    res = bass_utils.run_bass_kernel_spmd(
        nc, in_maps, core_ids=list(range(B)), trace=trace, **kw)
    out = np.stack([res.results[b]["out"] for b in range(B)], axis=0)
    return out, res


def kernel(**inputs):
    k = np.asarray(inputs["k"])
    v = np.asarray(inputs["v"])
    trace = bool(int(os.environ.get("KERNEL_TRACE", "0")))
    try:
        out, _ = _run(k, v, trace=trace)
    except Exception:
        # transient device hiccups happen; one retry on a fresh attempt
        out, _ = _run(k, v, trace=trace)
    return out.astype(np.float32)


# revision 40
# speedup vs baseline: 1.0119x; 1.0119x over previous
"""Laplace attention kernel for Trainium2 (8 NeuronCores, SPMD data-parallel).

Reference computation (per batch b):
    unnorm[i,j] = sum_d |(k[j,d] - v[i,d]) * 0.5|
    weights     = softmax_j(unnorm)          # rows i, softmax over j
    out[i,:]    = sum_j weights[i,j] * v[j,:]

B=8 batches -> one batch per NeuronCore, no cross-core communication.

Per-core algorithm (M=512, D=64, P=128):
  - Layouts:  vT2 [128=(t,d), 512=i] fp16 : v transposed, duplicated over t
              k2T [128=(t,d), 256=mj] f32 : column mj = [k[2mj,:]; k[2mj+1,:]]
  - For each j-pair mj: one DVE tensor_scalar
        absd[(t,d), i] = max(vT2, k2T[:,mj]) = max(v[i,d], k[2mj+t,d])
    then one TensorE matmul with a constant selector lhsT [128,2]
    (column t selects the 64 d-rows of half t) reducing over d:
        unnT[2m+t, i] += ... -> PSUM bank q holds unnT rows j=128q..128q+127
    |a-b| = 2*max(a,b) - a - b; the V1[i] part cancels in the softmax and
    the K1[j] part folds into the exp bias.  unnT is produced TRANSPOSED
    ([j,i]) which is exactly the lhsT the final matmul needs.
  - Producer split: banks 0..2 on VectorE (tensor_scalar max), bank 3 on
    ScalarE as Relu(v - k) = max(v,k) - k (bias absorbs the K1 sign flip).
    PE stream: 64 groups of (3 DVE-fed + 1 ScalarE-prebuffered) matmuls.
  - Softmax numerators wT[j,i] = exp(unnT - 0.5*K1[j] - SHIFT) in bf16.
  - Final matmul with v augmented by a ones column gives numerator and
    denominator together; one strided reciprocal + one broadcast multiply
    normalizes all 4 row-blocks; single DMA out.

Edge scheduling (v2): input DMAs issued from the gpsimd (k) and
vector+scalar (v halves) queues which come up earliest; PE warmed by dummy
matmuls on a memset tile until the transposes can start; all drain work
single-shot to cut the serial tail.
"""

import os

import numpy as np

M = 512
D = 64
B = 8
P = 128
NB = M // P  # 4 row-blocks
NMJ = M // 2  # 256 j-pairs
# Global shift on the softmax logits: weights are stored as
# exp(logit - EXP_SHIFT); numerator and denominator scale identically.
EXP_SHIFT = 38.0

_CACHE = {}

CFG = {"mx_dt": "float16"}


def _build_module(cfg=None):
    import concourse.mybir as mybir
    import concourse.tile as tile
    from concourse import bacc

    nc = bacc.Bacc("TRN2", target_bir_lowering=False, debug=False,
                   enable_asserts=False)
    k_dram = nc.dram_tensor("k", [M, D], mybir.dt.float32, kind="ExternalInput")
    v_dram = nc.dram_tensor("v", [M, D], mybir.dt.float32, kind="ExternalInput")
    out_dram = nc.dram_tensor("out", [M, D], mybir.dt.float32,
                              kind="ExternalOutput")

    with tile.TileContext(nc) as tc:
        _emit(tc, nc, k_dram.ap(), v_dram.ap(), out_dram.ap(), cfg or CFG)
    nc.compile()
    return nc


def _emit(tc, nc, k, v, out, cfg):
    from contextlib import ExitStack

    import concourse.mybir as mybir
    from concourse.masks import make_identity

    f32 = mybir.dt.float32
    fp16 = getattr(mybir.dt, cfg.get("mx_dt", "float16"))
    bf16 = mybir.dt.bfloat16
    Alu = mybir.AluOpType
    Act = mybir.ActivationFunctionType

    ctx = ExitStack()
    const = ctx.enter_context(tc.tile_pool(name="const", bufs=1))
    # Deep rings: DVE produces at ~262 ns/tile, PE consumes at ~200; buffering
    # lets the PE run at its native rate.  ScalarE prebuffers all of bank 3.
    absd_pool = ctx.enter_context(tc.tile_pool(name="absd", bufs=48))
    act_pool = ctx.enter_context(tc.tile_pool(name="absd_act", bufs=66))
    wt_pool = ctx.enter_context(tc.tile_pool(name="wt", bufs=4))
    small = ctx.enter_context(tc.tile_pool(name="small", bufs=1))
    # All three PSUM pools stay alive together (3 + 4 + 1 = 8 banks), so
    # the main accumulation never waits on the transpose pool's release.
    psum_tr = ctx.enter_context(tc.tile_pool(name="psum_tr", bufs=1,
                                             space="PSUM"))

    # ---- input DMAs first, at top scheduling priority --------------------
    # k whole on the gpsimd SWDGE path (issues earliest, 256 fat
    # descriptors, lands ~0.5us before v); v split across the two HWDGE
    # queues so its 512 thin descriptors transfer in parallel.
    v4 = const.tile([P, NB, D], f32, name="v4")
    v_view = v.rearrange("(q p) d -> p q d", p=P)
    k2_view = k.rearrange("(h m t) d -> m h (t d)", t=2, h=2)  # [128, 2, 128]
    k2all = const.tile([P, 2, P], f32, name="k2all")
    with tc.high_priority():
        nc.gpsimd.dma_start(k2all[:], k2_view[:])
        nc.sync.dma_start(v4[:, 0:2, :], v_view[:, 0:2, :])
        nc.scalar.dma_start(v4[:, 2:4, :], v_view[:, 2:4, :])

    # ---- warm-up source for PE p-state ramp ------------------------------
    warm_src = const.tile([P, P], fp16, name="warm_src")
    nc.gpsimd.memset(warm_src[:], 0.0)

    # ---- static tensors (gpsimd, after the DMA issues) -------------------
    # band[c, y] = 1 iff y == 64 + (c >= 64).  lhsT for local pair m is
    # band[:, 64-2m : 128-2m]: column p is 1 exactly when p == 2m + t(c).
    band = const.tile([P, 132], fp16, name="band")
    nc.gpsimd.memset(band[:], 0.0)
    nc.gpsimd.memset(band[0:D, D:D + 1], 1.0)
    nc.gpsimd.memset(band[D:2 * D, D + 1:D + 2], 1.0)
    ident = const.tile([P, P], f32, name="ident")
    make_identity(nc, ident)
    ident16 = const.tile([P, P], fp16, name="ident16")
    make_identity(nc, ident16)

    # ---- PE p-state warmup while DMAs are in flight ----------------------
    warm = psum_tr.tile([1, P], f32, name="warm", tag="warm")
    for _ in range(8):
        nc.tensor.matmul(warm[:], warm_src[:, 0:1], warm_src[:, 0:P],
                         start=True, stop=True)

    # ---- vT2 [128=(t,d), 512=i] fp16 (v lands first; PE does v first) ----
    # Everything split per DMA half / i-half so cast, transpose and the
    # duplicating copies pipeline instead of serializing.
    vT2 = const.tile([P, M], fp16, name="vT2")
    v16 = const.tile([P, NB, D], fp16, name="v16")
    ptv = psum_tr.tile([D, M], fp16, name="ptv", tag="ptv")
    for half in range(2):
        q0 = 2 * half
        nc.vector.tensor_copy(
            v16[:, q0:q0 + 2, :].rearrange("p q d -> p (q d)"),
            v4[:, q0:q0 + 2, :].rearrange("p q d -> p (q d)"))
        for q in (q0, q0 + 1):
            nc.tensor.transpose(ptv[:, q * P:(q + 1) * P], v16[:, q, :],
                                ident16[:])
    for half in range(2):
        sl = slice(half * 2 * P, (half + 1) * 2 * P)
        nc.vector.tensor_copy(vT2[0:D, sl], ptv[:, sl])
        nc.vector.tensor_copy(vT2[D:2 * D, sl], ptv[:, sl])

    # ---- k2T [128=(t,d), 256=mj] f32 -------------------------------------
    # Low half (banks 0,1 scalars for DVE) copied on DVE, high half on
    # ScalarE which then only negates the bank-3 columns it needs for the
    # Relu bias before starting its Relu stream.  tile_wait_until keeps the
    # scheduler from parking the k transposes at the PE queue head where
    # they would block the earlier-ready v transposes on the late k DMA.
    k2T = const.tile([P, NMJ], f32, name="k2T")
    ptrk = psum_tr.tile([P, 2 * P], f32, name="ptrk", tag="ptrk")
    with tc.tile_wait_until(0.0050):
        for h in range(2):
            nc.tensor.transpose(ptrk[:, h * P:(h + 1) * P], k2all[:, h, :],
                                ident[:])
        nc.vector.tensor_copy(k2T[:, 0:P], ptrk[:, 0:P])
        nc.scalar.copy(k2T[:, P:2 * P], ptrk[:, P:2 * P])
        neg_k2T = const.tile([P, D], f32, name="neg_k2T")  # mj 192..255
        nc.scalar.mul(neg_k2T[:], k2T[:, 3 * D:4 * D], -1.0)

    # ---- K1[j] = sum_d k[j,d] --------------------------------------------
    # Computed on ScalarE via activation accum_out (emitted inside the Relu
    # stream below, where ScalarE has slack); tiny scatter DMAs then build
    # the j-major per-bank bias columns.  Nothing touches DVE's hot path.
    k1m = const.tile([P, 2, 2], f32, name="k1m")
    k1scr = const.tile([P, D], fp16, name="k1scr")

    def emit_k1():
        for h in range(2):
            for t in range(2):
                nc.scalar.activation(k1scr[:], k2all[:, h, t * D:(t + 1) * D],
                                     Act.Copy, accum_out=k1m[:, h, t:t + 1])
        for q in range(NB):
            nc.sync.dma_start(k1_cols[:, q:q + 1],
                              k1m[(q % 2) * D:(q % 2) * D + D, q // 2, :])

    k1_cols = const.tile([P, NB], f32, name="k1_cols")

    # ---- main-phase PSUM pools -------------------------------------------
    psum_unn = ctx.enter_context(tc.tile_pool(name="psum_unn", bufs=4,
                                              space="PSUM"))
    psum_out = ctx.enter_context(tc.tile_pool(name="psum_out", bufs=1,
                                              space="PSUM"))
    out_all = psum_out.tile([P, NB, D + 1], f32, name="out_all")
    unns = [None] * NB
    for q in range(NB):
        unns[q] = psum_unn.tile([P, M], f32, name=f"unn_{q}", tag="unn")

    bias_col = [None] * NB
    wts = [None] * NB

    def emit_bias():
        for q in range(NB):
            bc = const.tile([P, 1], f32, name=f"bias_{q}")
            sgn = 0.5 if q == NB - 1 else -0.5  # bank 3 is the Relu path
            nc.scalar.activation(bc[:], k1_cols[:, q:q + 1], Act.Copy,
                                 bias=-EXP_SHIFT, scale=sgn)
            bias_col[q] = bc

    v_aug = []

    def emit_v_aug():
        for q in range(NB):
            va = const.tile([P, D + 1], bf16, name=f"v_aug_{q}")
            nc.scalar.copy(va[:, 0:D], v4[:, q, :])
            nc.gpsimd.memset(va[:, D:D + 1], 1.0)
            v_aug.append(va)

    def emit_exp(q, chunks=1):
        wT = wt_pool.tile([P, M], bf16, name="wT", tag="wT")
        wts[q] = wT
        cw = M // chunks
        for c in range(chunks):
            nc.scalar.activation(wT[:, c * cw:(c + 1) * cw],
                                 unns[q][:, c * cw:(c + 1) * cw],
                                 Act.Exp, scale=1.0, bias=bias_col[q][:])

    # ---- bank-3 distance tiles: ScalarE emits all 64 up front, with the
    # drain-phase helpers slotted into its queue where they have slack ----
    absd_a_tiles = {}
    for step in range(64):
        h, m = step % 2, step // 2
        mjl = h * 32 + m                # local col within the bank-3 block
        absd = act_pool.tile([P, M], fp16, name="absd_a", tag="absd_a")
        nc.scalar.activation(absd[:], vT2[:], Act.Relu,
                             bias=neg_k2T[:, mjl:mjl + 1], scale=1.0)
        absd_a_tiles[step] = absd
        if step == 3:
            emit_v_aug()
        elif step == 6:
            emit_k1()
        elif step == 16:
            emit_bias()

    def emit_step(q, step, absd):
        h, m = step % 2, step // 2
        nc.tensor.matmul(
            unns[q][D * h:D * h + D, :], band[:, D - 2 * m:2 * D - 2 * m],
            absd[:], start=(m == 0), stop=(m == 31), skip_group_check=True)

    # PE stream: groups of (3 VectorE-fed + 1 ScalarE-prebuffered) matmuls.
    # First group is all-DVE (ScalarE's first Relu tile lands late in the
    # startup chain).  The last group is all-DVE with group 62 taking three
    # prebuffered ScalarE tiles, so bank 3 closes ~0.8us before the stream
    # ends and its exp overlaps the last matmuls.
    sched = ["D"] * 4
    for g in range(1, 62):
        sched += ["D", "D", "D", "S"]
    sched += ["D", "S", "S", "S"]
    sched += ["D"] * 4
    ds = 0
    ss = 0
    for kind in sched:
        if kind == "D":
            q, step = ds // 64, ds % 64
            ds += 1
            h, m = step % 2, step // 2
            mj = q * 64 + h * 32 + m
            absd = absd_pool.tile([P, M], fp16, name="absd", tag="absd")
            nc.vector.tensor_scalar(
                absd[:], vT2[:], k2T[:, mj:mj + 1], None, op0=Alu.max)
            emit_step(q, step, absd)
        else:
            emit_step(NB - 1, ss, absd_a_tiles[ss])
            ss += 1

    # ---- softmax numerators ----------------------------------------------
    # Bank 3 closes early (schedule above) so its exp overlaps the last
    # matmuls; bank 2's exp is the only one that trails the stream.
    emit_exp(0)
    emit_exp(1)
    emit_exp(3)
    emit_exp(2)

    # ---- weighted sum + denominator via augmented-ones column ------------
    for qp in range(NB):
        for q in range(NB):
            nc.tensor.matmul(
                out_all[:, qp, :], wts[q][:, qp * P:(qp + 1) * P],
                v_aug[q][:], start=(q == 0), stop=(q == NB - 1),
                skip_group_check=True)

    # ---- normalize (single shot) + single DMA out ------------------------
    recip = small.tile([P, NB], f32, name="recip")
    nc.vector.reciprocal(recip[:], out_all[:, :, D])
    res = small.tile([P, NB, D], f32, name="res")
    rb = recip[:].unsqueeze(2).broadcast_to((P, NB, D))
    nc.vector.tensor_tensor(res[:], out_all[:, :, 0:D], rb, op=Alu.mult)
    out_v = out.rearrange("(q p) d -> p q d", p=P)
    nc.sync.dma_start(out_v[:, 0:2, :], res[:, 0:2, :])
    nc.scalar.dma_start(out_v[:, 2:4, :], res[:, 2:4, :])

    ctx.close()


def _get_module():
    if "nc" not in _CACHE:
        _CACHE["nc"] = _build_module()
    return _CACHE["nc"]


def _run(k, v, trace=False, tmpdir=None):
    """k, v: [B, M, D] f32. Returns (out [B, M, D] f32, BassKernelResults)."""
    from concourse import bass_utils

    nc = _get_module()
    kw = {"tmpdir": tmpdir} if tmpdir else {}
    in_maps = [
        {"k": np.ascontiguousarray(k[b], dtype=np.float32),
         "v": np.ascontiguousarray(v[b], dtype=np.float32)}
        for b in range(B)
    ]
    res = bass_utils.run_bass_kernel_spmd(
        nc, in_maps, core_ids=list(range(B)), trace=trace, **kw)
    out = np.stack([res.results[b]["out"] for b in range(B)], axis=0)
    return out, res


def kernel(**inputs):
    k = np.asarray(inputs["k"])
    v = np.asarray(inputs["v"])
    trace = bool(int(os.environ.get("KERNEL_TRACE", "0")))
    try:
        out, _ = _run(k, v, trace=trace)
    except Exception:
        # transient device hiccups happen; one retry on a fresh attempt
        out, _ = _run(k, v, trace=trace)
    return out.astype(np.float32)


# revision 41
# speedup vs baseline: 1.0194x; 1.0074x over previous
"""Laplace attention kernel for Trainium2 (8 NeuronCores, SPMD data-parallel).

Reference computation (per batch b):
    unnorm[i,j] = sum_d |(k[j,d] - v[i,d]) * 0.5|
    weights     = softmax_j(unnorm)          # rows i, softmax over j
    out[i,:]    = sum_j weights[i,j] * v[j,:]

B=8 batches -> one batch per NeuronCore, no cross-core communication.

Per-core algorithm (M=512, D=64, P=128):
  - Layouts:  vT2 [128=(t,d), 512=i] fp16 : v transposed, duplicated over t
              k2T [128=(t,d), 256=mj] f32 : column mj = [k[2mj,:]; k[2mj+1,:]]
  - For each j-pair mj: one DVE tensor_scalar
        absd[(t,d), i] = max(vT2, k2T[:,mj]) = max(v[i,d], k[2mj+t,d])
    then one TensorE matmul with a constant selector lhsT [128,2]
    (column t selects the 64 d-rows of half t) reducing over d:
        unnT[2m+t, i] += ... -> PSUM bank q holds unnT rows j=128q..128q+127
    |a-b| = 2*max(a,b) - a - b; the V1[i] part cancels in the softmax and
    the K1[j] part folds into the exp bias.  unnT is produced TRANSPOSED
    ([j,i]) which is exactly the lhsT the final matmul needs.
  - Producer split: banks 0..2 on VectorE (tensor_scalar max), bank 3 on
    ScalarE as Relu(v - k) = max(v,k) - k (bias absorbs the K1 sign flip).
    PE stream: 64 groups of (3 DVE-fed + 1 ScalarE-prebuffered) matmuls.
  - Softmax numerators wT[j,i] = exp(unnT - 0.5*K1[j] - SHIFT) in bf16.
  - Final matmul with v augmented by a ones column gives numerator and
    denominator together; one strided reciprocal + one broadcast multiply
    normalizes all 4 row-blocks; single DMA out.

Edge scheduling: input DMAs issued at top priority (k whole on the gpsimd
SWDGE path, v halves on the two HWDGE queues); PE warmed by dummy matmuls
on a memset tile until the transposes can start; all three PSUM pools
disjoint (3+4+1 = 8 banks) so the main accumulation never waits on the
transpose pool's release; bank 3 closes early in the stream so its exp
overlaps the last matmuls; single strided reciprocal + broadcast multiply
+ two parallel half DMAs for the store.

Measured ~70.0us/core (from the 72.2us baseline).  The 50.5us main stream
is at the fp16 PE rhs-streaming floor (cost model: 512 cols x 1 cycle @
~2.5GHz per matmul); the ~9.7us post-DMA teardown and ~4us DMA-in latency
are fixed runtime costs, verified with a minimal do-nothing kernel
(14.8us).  fp8 DoubleRow (2x PE) was analyzed and rejected: e4m3's ~2.5%
relative error on O(1) operands gives ~0.19 logit rms, ~10x over the
tolerance budget even for a single d-slice.
"""

import os

import numpy as np

M = 512
D = 64
B = 8
P = 128
NB = M // P  # 4 row-blocks
NMJ = M // 2  # 256 j-pairs
# Global shift on the softmax logits: weights are stored as
# exp(logit - EXP_SHIFT); numerator and denominator scale identically.
EXP_SHIFT = 38.0

_CACHE = {}

CFG = {"mx_dt": "float16"}


def _build_module(cfg=None):
    import concourse.mybir as mybir
    import concourse.tile as tile
    from concourse import bacc

    nc = bacc.Bacc("TRN2", target_bir_lowering=False, debug=False,
                   enable_asserts=False)
    k_dram = nc.dram_tensor("k", [M, D], mybir.dt.float32, kind="ExternalInput")
    v_dram = nc.dram_tensor("v", [M, D], mybir.dt.float32, kind="ExternalInput")
    out_dram = nc.dram_tensor("out", [M, D], mybir.dt.float32,
                              kind="ExternalOutput")

    with tile.TileContext(nc) as tc:
        _emit(tc, nc, k_dram.ap(), v_dram.ap(), out_dram.ap(), cfg or CFG)
    nc.compile()
    return nc


def _emit(tc, nc, k, v, out, cfg):
    from contextlib import ExitStack

    import concourse.mybir as mybir
    from concourse.masks import make_identity

    f32 = mybir.dt.float32
    fp16 = getattr(mybir.dt, cfg.get("mx_dt", "float16"))
    bf16 = mybir.dt.bfloat16
    Alu = mybir.AluOpType
    Act = mybir.ActivationFunctionType

    ctx = ExitStack()
    const = ctx.enter_context(tc.tile_pool(name="const", bufs=1))
    # Deep rings: DVE produces at ~262 ns/tile, PE consumes at ~200; buffering
    # lets the PE run at its native rate.  ScalarE prebuffers all of bank 3.
    absd_pool = ctx.enter_context(tc.tile_pool(name="absd", bufs=48))
    act_pool = ctx.enter_context(tc.tile_pool(name="absd_act", bufs=66))
    wt_pool = ctx.enter_context(tc.tile_pool(name="wt", bufs=4))
    small = ctx.enter_context(tc.tile_pool(name="small", bufs=1))
    # All three PSUM pools stay alive together (3 + 4 + 1 = 8 banks), so
    # the main accumulation never waits on the transpose pool's release.
    psum_tr = ctx.enter_context(tc.tile_pool(name="psum_tr", bufs=1,
                                             space="PSUM"))

    # ---- input DMAs first, at top scheduling priority --------------------
    # k whole on the gpsimd SWDGE path (issues earliest, 256 fat
    # descriptors, lands ~0.5us before v); v split across the two HWDGE
    # queues so its 512 thin descriptors transfer in parallel.
    v4 = const.tile([P, NB, D], f32, name="v4")
    v_view = v.rearrange("(q p) d -> p q d", p=P)
    k2_view = k.rearrange("(h m t) d -> m h (t d)", t=2, h=2)  # [128, 2, 128]
    k2all = const.tile([P, 2, P], f32, name="k2all")
    with tc.high_priority():
        nc.gpsimd.dma_start(k2all[:], k2_view[:])
        nc.sync.dma_start(v4[:, 0:2, :], v_view[:, 0:2, :])
        nc.scalar.dma_start(v4[:, 2:4, :], v_view[:, 2:4, :])

    # ---- warm-up source for PE p-state ramp ------------------------------
    warm_src = const.tile([P, P], fp16, name="warm_src")
    nc.gpsimd.memset(warm_src[:], 0.0)

    # ---- static tensors (gpsimd, after the DMA issues) -------------------
    # band[c, y] = 1 iff y == 64 + (c >= 64).  lhsT for local pair m is
    # band[:, 64-2m : 128-2m]: column p is 1 exactly when p == 2m + t(c).
    band = const.tile([P, 132], fp16, name="band")
    nc.gpsimd.memset(band[:], 0.0)
    nc.gpsimd.memset(band[0:D, D:D + 1], 1.0)
    nc.gpsimd.memset(band[D:2 * D, D + 1:D + 2], 1.0)
    ident = const.tile([P, P], f32, name="ident")
    make_identity(nc, ident)
    ident16 = const.tile([P, P], fp16, name="ident16")
    make_identity(nc, ident16)

    # ---- PE p-state warmup while DMAs are in flight ----------------------
    warm = psum_tr.tile([1, P], f32, name="warm", tag="warm")
    for _ in range(8):
        nc.tensor.matmul(warm[:], warm_src[:, 0:1], warm_src[:, 0:P],
                         start=True, stop=True)

    # ---- vT2 [128=(t,d), 512=i] fp16 (v lands first; PE does v first) ----
    # Everything split per DMA half / i-half so cast, transpose and the
    # duplicating copies pipeline instead of serializing.
    vT2 = const.tile([P, M], fp16, name="vT2")
    v16 = const.tile([P, NB, D], fp16, name="v16")
    ptv = psum_tr.tile([D, M], fp16, name="ptv", tag="ptv")
    for half in range(2):
        q0 = 2 * half
        nc.vector.tensor_copy(
            v16[:, q0:q0 + 2, :].rearrange("p q d -> p (q d)"),
            v4[:, q0:q0 + 2, :].rearrange("p q d -> p (q d)"))
        for q in (q0, q0 + 1):
            nc.tensor.transpose(ptv[:, q * P:(q + 1) * P], v16[:, q, :],
                                ident16[:])
    for half in range(2):
        sl = slice(half * 2 * P, (half + 1) * 2 * P)
        nc.vector.tensor_copy(vT2[0:D, sl], ptv[:, sl])
        nc.vector.tensor_copy(vT2[D:2 * D, sl], ptv[:, sl])

    # ---- k2T [128=(t,d), 256=mj] f32 -------------------------------------
    # Low half (banks 0,1 scalars for DVE) copied on DVE, high half on
    # ScalarE which then only negates the bank-3 columns it needs for the
    # Relu bias before starting its Relu stream.  tile_wait_until keeps the
    # scheduler from parking the k transposes at the PE queue head where
    # they would block the earlier-ready v transposes on the late k DMA.
    k2T = const.tile([P, NMJ], f32, name="k2T")
    ptrk = psum_tr.tile([P, 2 * P], f32, name="ptrk", tag="ptrk")
    with tc.tile_wait_until(0.0050):
        for h in range(2):
            nc.tensor.transpose(ptrk[:, h * P:(h + 1) * P], k2all[:, h, :],
                                ident[:])
        nc.vector.tensor_copy(k2T[:, 0:P], ptrk[:, 0:P])
        nc.scalar.copy(k2T[:, P:2 * P], ptrk[:, P:2 * P])
        neg_k2T = const.tile([P, D], f32, name="neg_k2T")  # mj 192..255
        nc.scalar.mul(neg_k2T[:], k2T[:, 3 * D:4 * D], -1.0)

    # ---- K1[j] = sum_d k[j,d] --------------------------------------------
    # Computed on ScalarE via activation accum_out (emitted inside the Relu
    # stream below, where ScalarE has slack); tiny scatter DMAs then build
    # the j-major per-bank bias columns.  Nothing touches DVE's hot path.
    k1m = const.tile([P, 2, 2], f32, name="k1m")
    k1scr = const.tile([P, D], fp16, name="k1scr")

    def emit_k1():
        for h in range(2):
            for t in range(2):
                nc.scalar.activation(k1scr[:], k2all[:, h, t * D:(t + 1) * D],
                                     Act.Copy, accum_out=k1m[:, h, t:t + 1])
        for q in range(NB):
            nc.sync.dma_start(k1_cols[:, q:q + 1],
                              k1m[(q % 2) * D:(q % 2) * D + D, q // 2, :])

    k1_cols = const.tile([P, NB], f32, name="k1_cols")

    # ---- main-phase PSUM pools -------------------------------------------
    psum_unn = ctx.enter_context(tc.tile_pool(name="psum_unn", bufs=4,
                                              space="PSUM"))
    psum_out = ctx.enter_context(tc.tile_pool(name="psum_out", bufs=1,
                                              space="PSUM"))
    out_all = psum_out.tile([P, NB, D + 1], f32, name="out_all")
    unns = [None] * NB
    for q in range(NB):
        unns[q] = psum_unn.tile([P, M], f32, name=f"unn_{q}", tag="unn")

    bias_col = [None] * NB
    wts = [None] * NB

    def emit_bias():
        for q in range(NB):
            bc = const.tile([P, 1], f32, name=f"bias_{q}")
            sgn = 0.5 if q == NB - 1 else -0.5  # bank 3 is the Relu path
            nc.scalar.activation(bc[:], k1_cols[:, q:q + 1], Act.Copy,
                                 bias=-EXP_SHIFT, scale=sgn)
            bias_col[q] = bc

    v_aug = []

    def emit_v_aug():
        for q in range(NB):
            va = const.tile([P, D + 1], bf16, name=f"v_aug_{q}")
            nc.scalar.copy(va[:, 0:D], v4[:, q, :])
            nc.gpsimd.memset(va[:, D:D + 1], 1.0)
            v_aug.append(va)

    def emit_exp(q, chunks=1):
        wT = wt_pool.tile([P, M], bf16, name="wT", tag="wT")
        wts[q] = wT
        cw = M // chunks
        for c in range(chunks):
            nc.scalar.activation(wT[:, c * cw:(c + 1) * cw],
                                 unns[q][:, c * cw:(c + 1) * cw],
                                 Act.Exp, scale=1.0, bias=bias_col[q][:])

    # ---- bank-3 distance tiles: ScalarE emits all 64 up front, with the
    # drain-phase helpers slotted into its queue where they have slack ----
    absd_a_tiles = {}
    for step in range(64):
        h, m = step % 2, step // 2
        mjl = h * 32 + m                # local col within the bank-3 block
        absd = act_pool.tile([P, M], fp16, name="absd_a", tag="absd_a")
        nc.scalar.activation(absd[:], vT2[:], Act.Relu,
                             bias=neg_k2T[:, mjl:mjl + 1], scale=1.0)
        absd_a_tiles[step] = absd
        if step == 3:
            emit_v_aug()
        elif step == 6:
            emit_k1()
        elif step == 16:
            emit_bias()

    def emit_step(q, step, absd):
        h, m = step % 2, step // 2
        nc.tensor.matmul(
            unns[q][D * h:D * h + D, :], band[:, D - 2 * m:2 * D - 2 * m],
            absd[:], start=(m == 0), stop=(m == 31), skip_group_check=True)

    # PE stream: groups of (3 VectorE-fed + 1 ScalarE-prebuffered) matmuls.
    # First group is all-DVE (ScalarE's first Relu tile lands late in the
    # startup chain).  The last group is all-DVE with group 62 taking three
    # prebuffered ScalarE tiles, so bank 3 closes ~0.8us before the stream
    # ends and its exp overlaps the last matmuls.
    sched = ["D"] * 4
    for g in range(1, 62):
        sched += ["D", "D", "D", "S"]
    sched += ["D", "S", "S", "S"]
    sched += ["D"] * 4
    ds = 0
    ss = 0
    for kind in sched:
        if kind == "D":
            q, step = ds // 64, ds % 64
            ds += 1
            h, m = step % 2, step // 2
            mj = q * 64 + h * 32 + m
            absd = absd_pool.tile([P, M], fp16, name="absd", tag="absd")
            nc.vector.tensor_scalar(
                absd[:], vT2[:], k2T[:, mj:mj + 1], None, op0=Alu.max)
            emit_step(q, step, absd)
        else:
            emit_step(NB - 1, ss, absd_a_tiles[ss])
            ss += 1

    # ---- softmax numerators ----------------------------------------------
    # Bank 3 closes early (schedule above) so its exp overlaps the last
    # matmuls; bank 2's exp is the only one that trails the stream.
    emit_exp(0)
    emit_exp(1)
    emit_exp(3)
    emit_exp(2)

    # ---- weighted sum + denominator via augmented-ones column ------------
    for qp in range(NB):
        for q in range(NB):
            nc.tensor.matmul(
                out_all[:, qp, :], wts[q][:, qp * P:(qp + 1) * P],
                v_aug[q][:], start=(q == 0), stop=(q == NB - 1),
                skip_group_check=True)

    # ---- normalize (single shot) + single DMA out ------------------------
    recip = small.tile([P, NB], f32, name="recip")
    nc.vector.reciprocal(recip[:], out_all[:, :, D])
    res = small.tile([P, NB, D], f32, name="res")
    rb = recip[:].unsqueeze(2).broadcast_to((P, NB, D))
    nc.vector.tensor_tensor(res[:], out_all[:, :, 0:D], rb, op=Alu.mult)
    out_v = out.rearrange("(q p) d -> p q d", p=P)
    nc.sync.dma_start(out_v[:, 0:2, :], res[:, 0:2, :])
    nc.scalar.dma_start(out_v[:, 2:4, :], res[:, 2:4, :])

    ctx.close()


def _get_module():
    if "nc" not in _CACHE:
        _CACHE["nc"] = _build_module()
    return _CACHE["nc"]


def _run(k, v, trace=False, tmpdir=None):
    """k, v: [B, M, D] f32. Returns (out [B, M, D] f32, BassKernelResults)."""
    from concourse import bass_utils

    nc = _get_module()
    kw = {"tmpdir": tmpdir} if tmpdir else {}
    in_maps = [
        {"k": np.ascontiguousarray(k[b], dtype=np.float32),
         "v": np.ascontiguousarray(v[b], dtype=np.float32)}
        for b in range(B)
    ]
    res = bass_utils.run_bass_kernel_spmd(
        nc, in_maps, core_ids=list(range(B)), trace=trace, **kw)
    out = np.stack([res.results[b]["out"] for b in range(B)], axis=0)
    return out, res


def kernel(**inputs):
    k = np.asarray(inputs["k"])
    v = np.asarray(inputs["v"])
    trace = bool(int(os.environ.get("KERNEL_TRACE", "0")))
    try:
        out, _ = _run(k, v, trace=trace)
    except Exception:
        # transient device hiccups happen; one retry on a fresh attempt
        out, _ = _run(k, v, trace=trace)
    return out.astype(np.float32)


# revision 50
# speedup vs baseline: 1.0203x; 1.0009x over previous
"""Laplace attention kernel for Trainium2 (8 NeuronCores, SPMD data-parallel).

Reference computation (per batch b):
    unnorm[i,j] = sum_d |(k[j,d] - v[i,d]) * 0.5|
    weights     = softmax_j(unnorm)          # rows i, softmax over j
    out[i,:]    = sum_j weights[i,j] * v[j,:]

B=8 batches -> one batch per NeuronCore, no cross-core communication.

Per-core algorithm (M=512, D=64, P=128):
  - Layouts:  vT2 [128=(t,d), 512=i] fp16 : v transposed, duplicated over t
              k2T [128=(t,d), 256=mj] f32 : column mj = [k[2mj,:]; k[2mj+1,:]]
  - For each j-pair mj: one DVE tensor_scalar
        absd[(t,d), i] = max(vT2, k2T[:,mj]) = max(v[i,d], k[2mj+t,d])
    then one TensorE matmul with a constant selector lhsT [128,2]
    (column t selects the 64 d-rows of half t) reducing over d:
        unnT[2m+t, i] += ... -> PSUM bank q holds unnT rows j=128q..128q+127
    |a-b| = 2*max(a,b) - a - b; the V1[i] part cancels in the softmax and
    the K1[j] part folds into the exp bias.  unnT is produced TRANSPOSED
    ([j,i]) which is exactly the lhsT the final matmul needs.
  - Producer split: banks 0..2 on VectorE (tensor_scalar max), bank 3 on
    ScalarE as Relu(v - k) = max(v,k) - k (bias absorbs the K1 sign flip).
    PE stream: 64 groups of (3 DVE-fed + 1 ScalarE-prebuffered) matmuls.
  - Softmax numerators wT[j,i] = exp(unnT - 0.5*K1[j] - SHIFT) in bf16.
  - Final matmul with v augmented by a ones column gives numerator and
    denominator together; one strided reciprocal + one broadcast multiply
    normalizes all 4 row-blocks; single DMA out.

Edge scheduling: input DMAs issued at top priority (k whole on the gpsimd
SWDGE path, v halves on the two HWDGE queues); PE warmed by dummy matmuls
on a memset tile until the transposes can start; all three PSUM pools
disjoint (3+4+1 = 8 banks) so the main accumulation never waits on the
transpose pool's release; bank 3 closes early in the stream so its exp
overlaps the last matmuls; single strided reciprocal + broadcast multiply
+ two parallel half DMAs for the store.

Measured ~70.0us/core (from the 72.2us baseline).  The 50.5us main stream
is at the fp16 PE rhs-streaming floor (cost model: 512 cols x 1 cycle @
~2.5GHz per matmul); the ~9.7us post-DMA teardown and ~4us DMA-in latency
are fixed runtime costs, verified with a minimal do-nothing kernel
(14.8us).  fp8 DoubleRow (2x PE) was analyzed and rejected: e4m3's ~2.5%
relative error on O(1) operands gives ~0.19 logit rms, ~10x over the
tolerance budget even for a single d-slice.
"""

import os

import numpy as np

M = 512
D = 64
B = 8
P = 128
NB = M // P  # 4 row-blocks
NMJ = M // 2  # 256 j-pairs
# Global shift on the softmax logits: weights are stored as
# exp(logit - EXP_SHIFT); numerator and denominator scale identically.
EXP_SHIFT = 38.0

_CACHE = {}

CFG = {"mx_dt": "float16"}


def _build_module(cfg=None):
    import concourse.mybir as mybir
    import concourse.tile as tile
    from concourse import bacc

    nc = bacc.Bacc("TRN2", target_bir_lowering=False, debug=False,
                   enable_asserts=False)
    k_dram = nc.dram_tensor("k", [M, D], mybir.dt.float32, kind="ExternalInput")
    v_dram = nc.dram_tensor("v", [M, D], mybir.dt.float32, kind="ExternalInput")
    out_dram = nc.dram_tensor("out", [M, D], mybir.dt.float32,
                              kind="ExternalOutput")

    with tile.TileContext(nc) as tc:
        _emit(tc, nc, k_dram.ap(), v_dram.ap(), out_dram.ap(), cfg or CFG)
    nc.compile()
    return nc


def _emit(tc, nc, k, v, out, cfg):
    from contextlib import ExitStack

    import concourse.mybir as mybir
    from concourse.masks import make_identity

    f32 = mybir.dt.float32
    fp16 = getattr(mybir.dt, cfg.get("mx_dt", "float16"))
    bf16 = mybir.dt.bfloat16
    Alu = mybir.AluOpType
    Act = mybir.ActivationFunctionType

    ctx = ExitStack()
    const = ctx.enter_context(tc.tile_pool(name="const", bufs=1))
    # Deep rings: DVE produces at ~262 ns/tile, PE consumes at ~200; buffering
    # lets the PE run at its native rate.  ScalarE prebuffers all of bank 3.
    absd_pool = ctx.enter_context(tc.tile_pool(name="absd", bufs=48))
    act_pool = ctx.enter_context(tc.tile_pool(name="absd_act", bufs=66))
    wt_pool = ctx.enter_context(tc.tile_pool(name="wt", bufs=4))
    small = ctx.enter_context(tc.tile_pool(name="small", bufs=1))
    # All three PSUM pools stay alive together (3 + 4 + 1 = 8 banks), so
    # the main accumulation never waits on the transpose pool's release.
    psum_tr = ctx.enter_context(tc.tile_pool(name="psum_tr", bufs=1,
                                             space="PSUM"))

    # ---- input DMAs first, at top scheduling priority --------------------
    # k whole on the gpsimd SWDGE path (issues earliest, 256 fat
    # descriptors, lands ~0.5us before v); v split across the two HWDGE
    # queues so its 512 thin descriptors transfer in parallel.
    v4 = const.tile([P, NB, D], f32, name="v4")
    v_view = v.rearrange("(q p) d -> p q d", p=P)
    k2_view = k.rearrange("(h m t) d -> m h (t d)", t=2, h=2)  # [128, 2, 128]
    k2all = const.tile([P, 2, P], f32, name="k2all")
    with tc.high_priority():
        nc.gpsimd.dma_start(k2all[:], k2_view[:])
        nc.sync.dma_start(v4[:, 0:2, :], v_view[:, 0:2, :])
        nc.scalar.dma_start(v4[:, 2:4, :], v_view[:, 2:4, :])

    # ---- warm-up source for PE p-state ramp ------------------------------
    warm_src = const.tile([P, P], fp16, name="warm_src")
    nc.gpsimd.memset(warm_src[:], 0.0)

    # ---- static tensors (gpsimd, after the DMA issues) -------------------
    # band[c, y] = 1 iff y == 64 + (c >= 64).  lhsT for local pair m is
    # band[:, 64-2m : 128-2m]: column p is 1 exactly when p == 2m + t(c).
    band = const.tile([P, 132], fp16, name="band")
    nc.gpsimd.memset(band[:], 0.0)
    nc.gpsimd.memset(band[0:D, D:D + 1], 1.0)
    nc.gpsimd.memset(band[D:2 * D, D + 1:D + 2], 1.0)
    ident = const.tile([P, P], f32, name="ident")
    make_identity(nc, ident)
    ident16 = const.tile([P, P], fp16, name="ident16")
    make_identity(nc, ident16)

    # ---- PE p-state warmup while DMAs are in flight ----------------------
    warm = psum_tr.tile([1, P], f32, name="warm", tag="warm")
    for _ in range(8):
        nc.tensor.matmul(warm[:], warm_src[:, 0:1], warm_src[:, 0:P],
                         start=True, stop=True)

    # ---- vT2 [128=(t,d), 512=i] fp16 (v lands first; PE does v first) ----
    # Everything split per DMA half / i-half so cast, transpose and the
    # duplicating copies pipeline instead of serializing.
    vT2 = const.tile([P, M], fp16, name="vT2")
    v16 = const.tile([P, NB, D], fp16, name="v16")
    ptv = psum_tr.tile([D, M], fp16, name="ptv", tag="ptv")
    for half in range(2):
        q0 = 2 * half
        nc.vector.tensor_copy(
            v16[:, q0:q0 + 2, :].rearrange("p q d -> p (q d)"),
            v4[:, q0:q0 + 2, :].rearrange("p q d -> p (q d)"))
        for q in (q0, q0 + 1):
            nc.tensor.transpose(ptv[:, q * P:(q + 1) * P], v16[:, q, :],
                                ident16[:])
    for half in range(2):
        sl = slice(half * 2 * P, (half + 1) * 2 * P)
        nc.vector.tensor_copy(vT2[0:D, sl], ptv[:, sl])
        nc.vector.tensor_copy(vT2[D:2 * D, sl], ptv[:, sl])

    # ---- k2T [128=(t,d), 256=mj] f32 -------------------------------------
    # Low half (banks 0,1 scalars for DVE) copied on DVE, high half on
    # ScalarE which then only negates the bank-3 columns it needs for the
    # Relu bias before starting its Relu stream.  tile_wait_until keeps the
    # scheduler from parking the k transposes at the PE queue head where
    # they would block the earlier-ready v transposes on the late k DMA.
    k2T = const.tile([P, NMJ], f32, name="k2T")
    ptrk = psum_tr.tile([P, 2 * P], f32, name="ptrk", tag="ptrk")
    with tc.tile_wait_until(0.0050):
        for h in range(2):
            nc.tensor.transpose(ptrk[:, h * P:(h + 1) * P], k2all[:, h, :],
                                ident[:])
        nc.vector.tensor_copy(k2T[:, 0:P], ptrk[:, 0:P])
        nc.scalar.copy(k2T[:, P:2 * P], ptrk[:, P:2 * P])
        neg_k2T = const.tile([P, D], f32, name="neg_k2T")  # mj 192..255
        nc.scalar.mul(neg_k2T[:], k2T[:, 3 * D:4 * D], -1.0)

    # ---- K1[j] = sum_d k[j,d] --------------------------------------------
    # Computed on ScalarE via activation accum_out (emitted inside the Relu
    # stream below, where ScalarE has slack); tiny scatter DMAs then build
    # the j-major per-bank bias columns.  Nothing touches DVE's hot path.
    k1m = const.tile([P, 2, 2], f32, name="k1m")
    k1scr = const.tile([P, D], fp16, name="k1scr")

    def emit_k1():
        for h in range(2):
            for t in range(2):
                nc.scalar.activation(k1scr[:], k2all[:, h, t * D:(t + 1) * D],
                                     Act.Copy, accum_out=k1m[:, h, t:t + 1])
        for q in range(NB):
            nc.sync.dma_start(k1_cols[:, q:q + 1],
                              k1m[(q % 2) * D:(q % 2) * D + D, q // 2, :])

    k1_cols = const.tile([P, NB], f32, name="k1_cols")

    # ---- main-phase PSUM pools -------------------------------------------
    psum_unn = ctx.enter_context(tc.tile_pool(name="psum_unn", bufs=4,
                                              space="PSUM"))
    psum_out = ctx.enter_context(tc.tile_pool(name="psum_out", bufs=1,
                                              space="PSUM"))
    out_all = psum_out.tile([P, NB, D + 1], f32, name="out_all")
    unns = [None] * NB
    for q in range(NB):
        unns[q] = psum_unn.tile([P, M], f32, name=f"unn_{q}", tag="unn")

    bias_col = [None] * NB
    wts = [None] * NB

    def emit_bias():
        for q in range(NB):
            bc = const.tile([P, 1], f32, name=f"bias_{q}")
            sgn = 0.5 if q == NB - 1 else -0.5  # bank 3 is the Relu path
            nc.scalar.activation(bc[:], k1_cols[:, q:q + 1], Act.Copy,
                                 bias=-EXP_SHIFT, scale=sgn)
            bias_col[q] = bc

    v_aug = []

    def emit_v_aug():
        for q in range(NB):
            va = const.tile([P, D + 1], bf16, name=f"v_aug_{q}")
            nc.scalar.copy(va[:, 0:D], v4[:, q, :])
            nc.gpsimd.memset(va[:, D:D + 1], 1.0)
            v_aug.append(va)

    def emit_exp(q, chunks=1):
        wT = wt_pool.tile([P, M], bf16, name="wT", tag="wT")
        wts[q] = wT
        cw = M // chunks
        for c in range(chunks):
            nc.scalar.activation(wT[:, c * cw:(c + 1) * cw],
                                 unns[q][:, c * cw:(c + 1) * cw],
                                 Act.Exp, scale=1.0, bias=bias_col[q][:])

    # ---- bank-3 distance tiles: ScalarE emits all 64 up front, with the
    # drain-phase helpers slotted into its queue where they have slack ----
    absd_a_tiles = {}
    for step in range(64):
        h, m = step % 2, step // 2
        mjl = h * 32 + m                # local col within the bank-3 block
        absd = act_pool.tile([P, M], fp16, name="absd_a", tag="absd_a")
        nc.scalar.activation(absd[:], vT2[:], Act.Relu,
                             bias=neg_k2T[:, mjl:mjl + 1], scale=1.0)
        absd_a_tiles[step] = absd
        if step == 3:
            emit_v_aug()
        elif step == 6:
            emit_k1()
        elif step == 16:
            emit_bias()

    def emit_step(q, step, absd):
        h, m = step % 2, step // 2
        nc.tensor.matmul(
            unns[q][D * h:D * h + D, :], band[:, D - 2 * m:2 * D - 2 * m],
            absd[:], start=(m == 0), stop=(m == 31), skip_group_check=True)

    # PE stream: groups of (3 VectorE-fed + 1 ScalarE-prebuffered) matmuls.
    # First group is all-DVE (ScalarE's first Relu tile lands late in the
    # startup chain).  The last group is all-DVE with group 62 taking three
    # prebuffered ScalarE tiles, so bank 3 closes ~0.8us before the stream
    # ends and its exp overlaps the last matmuls.
    sched = ["D"] * 4
    for g in range(1, 62):
        sched += ["D", "D", "D", "S"]
    sched += ["D", "S", "S", "S"]
    sched += ["D"] * 4
    ds = 0
    ss = 0
    for kind in sched:
        if kind == "D":
            q, step = ds // 64, ds % 64
            ds += 1
            h, m = step % 2, step // 2
            mj = q * 64 + h * 32 + m
            absd = absd_pool.tile([P, M], fp16, name="absd", tag="absd")
            nc.vector.tensor_scalar(
                absd[:], vT2[:], k2T[:, mj:mj + 1], None, op0=Alu.max)
            emit_step(q, step, absd)
        else:
            emit_step(NB - 1, ss, absd_a_tiles[ss])
            ss += 1

    # ---- softmax numerators ----------------------------------------------
    # Bank 3 closes early (schedule above) so its exp overlaps the last
    # matmuls; bank 2's exp is the only one that trails the stream.
    emit_exp(0)
    emit_exp(1)
    emit_exp(3)
    emit_exp(2)

    # ---- weighted sum + denominator via augmented-ones column ------------
    for qp in range(NB):
        for q in range(NB):
            nc.tensor.matmul(
                out_all[:, qp, :], wts[q][:, qp * P:(qp + 1) * P],
                v_aug[q][:], start=(q == 0), stop=(q == NB - 1),
                skip_group_check=True)

    # ---- normalize (single shot) + single DMA out ------------------------
    recip = small.tile([P, NB], f32, name="recip")
    nc.vector.reciprocal(recip[:], out_all[:, :, D])
    res = small.tile([P, NB, D], f32, name="res")
    rb = recip[:].unsqueeze(2).broadcast_to((P, NB, D))
    nc.vector.tensor_tensor(res[:], out_all[:, :, 0:D], rb, op=Alu.mult)
    out_v = out.rearrange("(q p) d -> p q d", p=P)
    nc.sync.dma_start(out_v[:, 0:2, :], res[:, 0:2, :])
    nc.scalar.dma_start(out_v[:, 2:4, :], res[:, 2:4, :])

    ctx.close()


def _get_module():
    if "nc" not in _CACHE:
        _CACHE["nc"] = _build_module()
    return _CACHE["nc"]


def _run(k, v, trace=False, tmpdir=None):
    """k, v: [B, M, D] f32. Returns (out [B, M, D] f32, BassKernelResults)."""
    from concourse import bass_utils

    nc = _get_module()
    kw = {"tmpdir": tmpdir} if tmpdir else {}
    in_maps = [
        {"k": np.ascontiguousarray(k[b], dtype=np.float32),
         "v": np.ascontiguousarray(v[b], dtype=np.float32)}
        for b in range(B)
    ]
    res = bass_utils.run_bass_kernel_spmd(
        nc, in_maps, core_ids=list(range(B)), trace=trace, **kw)
    out = np.stack([res.results[b]["out"] for b in range(B)], axis=0)
    return out, res


def kernel(**inputs):
    k = np.asarray(inputs["k"])
    v = np.asarray(inputs["v"])
    trace = bool(int(os.environ.get("KERNEL_TRACE", "0")))
    try:
        out, _ = _run(k, v, trace=trace)
    except Exception:
        # transient device hiccups happen; one retry on a fresh attempt
        out, _ = _run(k, v, trace=trace)
    return out.astype(np.float32)
